# revision 33
# baseline (speedup 1.0000x reference)
"""DepthwiseXCorr (SiamRPN-style depthwise cross-correlation head) on 8 trn2 cores.

Data-parallel over batch: B=128 -> 16 samples per core. Per sample:
  branch(x) = BN2(pw1x1(ReLU6(BN1(dw3x3(x)))))   for kernel (7x7) and search (31x31)
  out = per-channel xcorr(search_feat 29x29, kernel_feat 5x5) -> 25x25

Three-engine split (v2):
  - search dw conv: PE matmuls with constant per-tap DIAGONAL fp16 weight
    matrices (BN1 folded host-side); PSUM accumulates the 9 taps; ACT evicts
    with bias; DVE clamps (relu6) writing fp16.
  - pw convs: PE fp16 matmuls (BN2 folded), ACT evicts with bias (search
    features to fp16, kernel feats K2 to f32 for use as per-partition scalars).
  - kernel dw conv: DVE stt chain in f32 (tiny).
  - xcorr 25 taps per channel-block: split three ways per the measured
    engine costs: 'a' taps = DVE scalar_tensor_tensor chain on an fp16 acc;
    'b' taps = ACT per-partition-scale product then DVE tensor_tensor add
    (fp16, 2x mode); 'd' taps = ACT product then PE identity-matmul
    accumulate into PSUM. Final: PE folds the DVE acc into PSUM, ACT evicts
    f32, DMA out.
"""

import numpy as np

import concourse.bass as bass
import concourse.mybir as mybir
from concourse.tile import TileContext
from concourse.bass_utils import run_bass_kernel_spmd

F32 = mybir.dt.float32
F16 = mybir.dt.float16
AF = mybir.ActivationFunctionType
OP = mybir.AluOpType

B, C, KH, SH, KK = 128, 256, 7, 31, 3
N_CORES = 8
BPC = B // N_CORES          # samples per core
G = C // 128                # channel blocks
EPS = 1e-5

HO_K, HO_S, HO_X = KH - 2, SH - 2, 25   # 5, 29, 25

# f32 param columns: [wdk 18 | bdk 2 | bds 2 | bpk 2 | bps 2]
O_WDK, O_BDK, O_BDS, O_BPK, O_BPS = 0, 18, 20, 22, 24
P_F32 = 26
# fp16 param columns: [dwdiag 18*128 | Ws 4*128 | Wk 4*128 | ident 128]
O_DIAG, O_WS, O_WK, O_ID = 0, 2304, 2816, 3328
P_F16 = 3456

# xcorr tap routing per channel block: counts of types, sum 25.
# 'a': DVE stt chain; 'b': ACT product + DVE tt-add; 'c': DVE ts product +
# PE psum-add; 'd': ACT product + PE psum-add.
NA, NB, NC, ND = 2, 2, 14, 7

_cache: dict = {}

LAST_RESULTS = None         # stash for test harness (exec_time_ns etc.)


def _fold_branch(dw_w, bn1, pw_w, pw_b, bn2):
    """Fold eval-mode BN params into conv weights/biases (host, numpy fp32)."""
    g1, b1, m1, v1 = bn1[0], bn1[1], bn1[2], bn1[3]
    inv1 = g1 / np.sqrt(v1 + EPS)
    shift1 = b1 - m1 * inv1
    dw = (dw_w[:, 0] * inv1[:, None, None]).reshape(C, 9).astype(np.float32)

    g2, b2, m2, v2 = bn2[0], bn2[1], bn2[2], bn2[3]
    inv2 = g2 / np.sqrt(v2 + EPS)
    shift2 = b2 - m2 * inv2
    W = (pw_w[:, :, 0, 0] * inv2[:, None]).astype(np.float32)   # (co, ci)
    bias2 = (pw_b * inv2 + shift2).astype(np.float32)

    lhsT = np.zeros((G, G, 128, 128), np.float32)
    for gi in range(G):
        for go in range(G):
            lhsT[gi, go] = W[go * 128:(go + 1) * 128, gi * 128:(gi + 1) * 128].T
    dw_blk = dw.reshape(G, 128, 9)
    b1_blk = shift1.astype(np.float32).reshape(G, 128, 1)
    b2_blk = bias2.reshape(G, 128, 1)
    return dw_blk, b1_blk, lhsT, b2_blk


def _split_waits(nc, keep=1):
    """This container's walrus accepts only one sync-wait per instruction."""
    import bass_rust

    n = 0
    for bb in nc.m.functions[0].blocks:
        out = []
        for ins in bb.instructions:
            si = ins.sync_info
            if si is not None and len(si.on_wait) > keep:
                waits = list(si.on_wait)
                for w in waits[:-keep]:
                    n += 1
                    ev = mybir.InstEventSemaphore(
                        name=f"antsplitw_{n}", ins=[], outs=[])
                    ev.engine = ins.engine
                    ev.sync_info = bass_rust.SyncInfo(on_wait=[w], on_update=[])
                    out.append(ev)
                ins.sync_info = bass_rust.SyncInfo(
                    on_wait=waits[-keep:], on_update=list(si.on_update))
            out.append(ins)
        bb.instructions = out
    return n


def _route():
    """Per-tap type list, length 25; first entry must be 'a' (chain init)."""
    out = []
    pools = {'a': NA, 'b': NB, 'c': NC, 'd': ND}
    order = ['a', 'd', 'b', 'c']
    i = 0
    while len(out) < 25:
        t = order[i % 4]
        if pools[t] > 0:
            pools[t] -= 1
            out.append(t)
        i += 1
        if all(v == 0 for v in pools.values()):
            break
    assert len(out) == 25 and out[0] == 'a'
    return out


def _build_nc():
    nc = bass.Bass()

    kern_h = nc.declare_dram_parameter("kern_in", [BPC, C, KH, KH], F32, isOutput=False)
    srch_h = nc.declare_dram_parameter("srch_in", [BPC, C, SH, SH], F32, isOutput=False)
    prm_h = nc.declare_dram_parameter("params", [128, P_F32], F32, isOutput=False)
    prh_h = nc.declare_dram_parameter("params16", [128, P_F16], F16, isOutput=False)
    out_h = nc.declare_dram_parameter("out", [BPC, C, 625], F32, isOutput=True)

    route = _route()

    with TileContext(nc) as tc:
        with (
            tc.tile_pool(name="const", bufs=1) as cpool,
            tc.tile_pool(name="kio", bufs=2) as kpool,
            tc.tile_pool(name="sio", bufs=2) as spool,
            tc.tile_pool(name="s16", bufs=2) as s16pool,
            tc.tile_pool(name="feat", bufs=3) as fpool,
            tc.tile_pool(name="prod", bufs=2) as prpool,
            tc.tile_pool(name="xout", bufs=2) as xpool,
            tc.tile_pool(name="psdw", bufs=1, space="PSUM") as pdw,
            tc.tile_pool(name="pspw", bufs=1, space="PSUM") as ppw,
            tc.tile_pool(name="psk", bufs=2, space="PSUM") as pk,
            tc.tile_pool(name="psx", bufs=1, space="PSUM") as px,
        ):
            prm = cpool.tile([128, P_F32], F32)
            nc.sync.dma_start(out=prm[:], in_=prm_h[:])
            prh = cpool.tile([128, P_F16], F16)
            nc.sync.dma_start(out=prh[:], in_=prh_h[:])

            def _b(base, g):          # f32 bias col [128,1]
                return prm[:, base + g:base + g + 1]

            def _diag(g, t):          # fp16 diag block [128,128]
                o = O_DIAG + (g * 9 + t) * 128
                return prh[:, o:o + 128]

            def _ws(gi, go):
                o = O_WS + (gi * G + go) * 128
                return prh[:, o:o + 128]

            def _wk(gi, go):
                o = O_WK + (gi * G + go) * 128
                return prh[:, o:o + 128]

            ident = prh[:, O_ID:O_ID + 128]

            def emit_front(b):
                # ---- kernel branch: dw conv on DVE (f32, tiny) ----
                hk = []
                for g in range(G):
                    xk = kpool.tile([128, KH, KH], F32, name="xk")
                    nc.sync.dma_start(out=xk[:], in_=kern_h[b, 128 * g:128 * (g + 1)])
                    acc = fpool.tile([128, HO_K, HO_K], F32, name=f"acck{g}")
                    nc.vector.tensor_scalar(
                        acc[:], xk[:, 0:HO_K, 0:HO_K],
                        prm[:, O_WDK + g * 9:O_WDK + g * 9 + 1], _b(O_BDK, g),
                        OP.mult, OP.add)
                    for t in range(1, 9):
                        u, v = t // 3, t % 3
                        nc.vector.scalar_tensor_tensor(
                            acc[:], xk[:, u:u + HO_K, v:v + HO_K],
                            prm[:, O_WDK + g * 9 + t:O_WDK + g * 9 + t + 1],
                            acc[:], OP.mult, OP.add)
                    h16 = fpool.tile([128, HO_K * HO_K], F16, name=f"hk{g}")
                    nc.vector.tensor_scalar(
                        h16[:], acc[:].rearrange("p a b -> p (a b)"), 6.0, 0.0,
                        OP.min, OP.max)
                    hk.append(h16)

                # ---- kernel branch pw conv (PE fp16) -> K2 f32 scalars ----
                K2 = fpool.tile([128, 2 * 25], F32, name="K2")
                for go in range(G):
                    kps = pk.tile([128, 25], F32, name="kps")
                    for gi in range(G):
                        nc.tensor.matmul(kps[:], _wk(gi, go), hk[gi][:],
                                         start=(gi == 0), stop=(gi == G - 1))
                    nc.scalar.activation(K2[:, go * 25:go * 25 + 25], kps[:],
                                         AF.Identity, bias=_b(O_BPK, go), scale=1.0)

                # ---- search branch ----
                hs = []
                for g in range(G):
                    xs = spool.tile([128, SH, SH], F32, name="xs")
                    nc.sync.dma_start(out=xs[:], in_=srch_h[b, 128 * g:128 * (g + 1)])
                    xs16 = s16pool.tile([128, SH, 32], F16, name="xs16")
                    nc.gpsimd.tensor_copy(xs16[:, 0:SH, 0:SH], xs[:])
                    # dw conv: 9 diag-matmul taps, rows split 15/14 across banks
                    dA = pdw.tile([128, 15 * HO_S], F32, name="dA")
                    dB = pdw.tile([128, 14 * HO_S], F32, name="dB")
                    for t in range(9):
                        u, v = t // 3, t % 3
                        nc.tensor.matmul(
                            dA[:], _diag(g, t), xs16[:, u:u + 15, v:v + HO_S],
                            start=(t == 0), stop=(t == 8))
                        nc.tensor.matmul(
                            dB[:], _diag(g, t), xs16[:, u + 15:u + 29, v:v + HO_S],
                            start=(t == 0), stop=(t == 8))
                    h = fpool.tile([128, HO_S, 32], F16, name=f"hs{g}")
                    nc.scalar.activation(h[:, 0:15, 0:HO_S], dA[:], AF.Relu,
                                         bias=_b(O_BDS, g), scale=1.0)
                    nc.scalar.activation(h[:, 15:29, 0:HO_S], dB[:], AF.Relu,
                                         bias=_b(O_BDS, g), scale=1.0)
                    hs.append(h)

                # ---- search pw conv (PE fp16) -> S2 fp16 ----
                S2 = []
                for go in range(G):
                    pA = ppw.tile([128, 17 * HO_S], F32, name="pA")
                    pB = ppw.tile([128, 12 * HO_S], F32, name="pB")
                    for gi in range(G):
                        nc.tensor.matmul(pA[:], _ws(gi, go),
                                         hs[gi][:, 0:17, 0:HO_S],
                                         start=(gi == 0), stop=(gi == G - 1))
                        nc.tensor.matmul(pB[:], _ws(gi, go),
                                         hs[gi][:, 17:29, 0:HO_S],
                                         start=(gi == 0), stop=(gi == G - 1))
                    s2 = fpool.tile([128, HO_S, 32], F16, name=f"s2_{go}")
                    nc.scalar.activation(s2[:, 0:17, 0:HO_S], pA[:], AF.Identity,
                                         bias=_b(O_BPS, go), scale=1.0)
                    nc.scalar.activation(s2[:, 17:29, 0:HO_S], pB[:], AF.Identity,
                                         bias=_b(O_BPS, go), scale=1.0)
                    S2.append(s2)
                return S2, K2

            def emit_xcorr(b, S2, K2):
                for g in range(G):
                    def win(t):
                        u, v = t // 5, t % 5
                        return S2[g][:, u:u + 25, v:v + 25]

                    def kcol(t):
                        return K2[:, g * 25 + t:g * 25 + t + 1]

                    # products: 'b'/'d' on ACT, 'c' on DVE
                    prods = {}
                    nslot = 0
                    for t in range(25):
                        if route[t] in ('b', 'c', 'd'):
                            p = prpool.tile([128, 625], F16,
                                            name=f"pr{g}_{nslot % 9}")
                            nslot += 1
                            if route[t] == 'c':
                                nc.vector.tensor_scalar(
                                    p[:].rearrange("p (a b) -> p a b", a=25),
                                    win(t), kcol(t), None, OP.mult)
                            else:
                                nc.scalar.activation(
                                    p[:].rearrange("p (a b) -> p a b", a=25),
                                    win(t), AF.Identity, bias=0.0, scale=kcol(t))
                            prods[t] = p

                    # DVE: init + stt chain for 'a' taps
                    acc = xpool.tile([128, 625], F16, name=f"xacc{g}")
                    first = True
                    for t in range(25):
                        if route[t] != 'a':
                            continue
                        if first:
                            nc.vector.tensor_scalar(
                                acc[:].rearrange("p (a b) -> p a b", a=25),
                                win(t), kcol(t), None, OP.mult)
                            first = False
                        else:
                            nc.vector.scalar_tensor_tensor(
                                acc[:].rearrange("p (a b) -> p a b", a=25),
                                win(t), kcol(t), acc[:].rearrange(
                                    "p (a b) -> p a b", a=25),
                                OP.mult, OP.add)
                    # DVE: tensor_tensor adds for 'b' taps
                    for t in range(25):
                        if route[t] == 'b':
                            nc.vector.tensor_tensor(acc[:], prods[t][:], acc[:],
                                                    OP.add)

                    # PE: identity-matmul accumulate for 'c'/'d' taps
                    xA = px.tile([128, 512], F32, name="xA")
                    xB = px.tile([128, 128], F32, name="xB")
                    d_taps = [t for t in range(25) if route[t] in ('c', 'd')]
                    for i, t in enumerate(d_taps):
                        st = (i == 0)
                        nc.tensor.matmul(xA[:], ident, prods[t][:, 0:512],
                                         start=st, stop=False)
                        nc.tensor.matmul(xB[:, 0:113], ident, prods[t][:, 512:625],
                                         start=st, stop=False)
                    # fold DVE acc into PSUM
                    nc.tensor.matmul(xA[:], ident, acc[:, 0:512],
                                     start=False, stop=True)
                    nc.tensor.matmul(xB[:, 0:113], ident, acc[:, 512:625],
                                     start=False, stop=True)

                    # ACT evicts PSUM -> SBUF f32, then DMA
                    of = xpool.tile([128, 625], F32, name=f"of{g}")
                    nc.scalar.activation(of[:, 0:512], xA[:], AF.Identity,
                                         bias=0.0, scale=1.0)
                    nc.scalar.activation(of[:, 512:625], xB[:, 0:113],
                                         AF.Identity, bias=0.0, scale=1.0)
                    nc.sync.dma_start(out=out_h[b, 128 * g:128 * (g + 1)],
                                      in_=of[:])

            # software pipeline: front(b) then xcorr(b-1), so each engine's
            # queue interleaves the two phases of adjacent samples.
            pend = None
            for b in range(BPC):
                cur = emit_front(b)
                if pend is not None:
                    emit_xcorr(b - 1, *pend)
                pend = cur
            emit_xcorr(BPC - 1, *pend)
    _split_waits(nc)
    return nc


def kernel(kernel, search, k_dw_w, k_bn1, k_pw_w, k_pw_b, k_bn2,
           s_dw_w, s_bn1, s_pw_w, s_pw_b, s_bn2):
    global LAST_RESULTS
    kdw, kb1, kpw, kb2 = _fold_branch(np.asarray(k_dw_w), np.asarray(k_bn1),
                                      np.asarray(k_pw_w), np.asarray(k_pw_b),
                                      np.asarray(k_bn2))
    sdw, sb1, spw, sb2 = _fold_branch(np.asarray(s_dw_w), np.asarray(s_bn1),
                                      np.asarray(s_pw_w), np.asarray(s_pw_b),
                                      np.asarray(s_bn2))
    kern = np.ascontiguousarray(np.asarray(kernel, np.float32))
    srch = np.ascontiguousarray(np.asarray(search, np.float32))

    if "nc" not in _cache:
        _cache["nc"] = _build_nc()
    nc = _cache["nc"]

    prm = np.zeros((128, P_F32), np.float32)
    # dw-k weights, per-partition: wdk[c_in_block, g*9+t]
    prm[:, O_WDK:O_WDK + 18] = kdw.transpose(1, 0, 2).reshape(128, 18)
    prm[:, O_BDK:O_BDK + G] = kb1.transpose(1, 0, 2).reshape(128, G)
    prm[:, O_BDS:O_BDS + G] = sb1.transpose(1, 0, 2).reshape(128, G)
    prm[:, O_BPK:O_BPK + G] = kb2.transpose(1, 0, 2).reshape(128, G)
    prm[:, O_BPS:O_BPS + G] = sb2.transpose(1, 0, 2).reshape(128, G)

    prh = np.zeros((128, P_F16), np.float16)
    for g in range(G):
        for t in range(9):
            o = O_DIAG + (g * 9 + t) * 128
            prh[:, o:o + 128] = np.diag(sdw[g][:, t]).astype(np.float16)
    prh[:, O_WS:O_WS + 512] = spw.transpose(2, 0, 1, 3).reshape(128, 512).astype(
        np.float16)
    prh[:, O_WK:O_WK + 512] = kpw.transpose(2, 0, 1, 3).reshape(128, 512).astype(
        np.float16)
    prh[:, O_ID:O_ID + 128] = np.eye(128, dtype=np.float16)

    in_maps = []
    for i in range(N_CORES):
        sl = slice(i * BPC, (i + 1) * BPC)
        in_maps.append({"kern_in": kern[sl], "srch_in": srch[sl],
                        "params": prm, "params16": prh})

    res = run_bass_kernel_spmd(nc, in_maps, list(range(N_CORES)))
    LAST_RESULTS = res
    out = np.concatenate([res.results[i]["out"] for i in range(N_CORES)], axis=0)
    return out.reshape(B, C, 25, 25)
